# revision 27
# baseline (speedup 1.0000x reference)
"""Single-layer transformer LM head kernel for 8 Trainium2 NeuronCores (v2).

Model (B=2, T=2048, D=1024, V=32000):
    x = tok_emb[idx] + pos_emb
    x = x + 0.125 * causal_attn(x@Wq, x@Wk, x@Wv)
    x = x + gelu(x@W1 + b1)@W2 + b2
    out = x@Wout + bout

Sharding (one uniform SPMD program on 8 cores):
  - trunk token-parallel: core c owns 512 tokens (batch c//4, block c%4);
    K/V for the whole batch-sequence are computed locally on every core from
    host-prepared fp8 embeddings, with the 512-token blocks ROTATED so the
    core's own block is always first (static/uniform access patterns;
    causality lives in per-core mask inputs).
  - final-hidden AllGather (bf16) across all 8 cores, split over token
    halves: the W2 loop computes tokens 0:256 first and AllGathers them
    while tokens 256:512 compute, so the logits phase starts stall-free.
  - logits vocab-parallel: each core does all 4096 tokens x 4000 vocab cols.

Precision strategy (validated vs reference, rel_err ~3.8e-3 << 2e-2 gate):
  - attention block entirely in fp8e4m3 with DoubleRow matmuls (2 contraction
    chunks per instruction, 2x PE rate): x8 = fp8(x*256), W fp8(W*512),
    q/k/v re-quantized to fp8(v*64) off PSUM, probs stored as fp8(16*exp);
    the softmax normalizer is computed from the same fp8 probs so prob
    quantization partially cancels. Residual path stays fp32.
  - MLP in bf16 (weights host-converted, activations bf16; residual fp32).
  - logits in bf16 (hidden states, Wout, and the output tensor itself are
    bf16; host upcasts). Halves weight DMA, collective, and output traffic.
K and V stay SBUF-resident in fp8 (no DRAM bounce): the projection PSUM
layouts already match what the scores / attn@V matmuls need.
"""
import numpy as np
import ml_dtypes
import concourse.bass as bass
import concourse.bacc as bacc
import concourse.tile as tile
from concourse import bass_utils, mybir

F32 = mybir.dt.float32
BF16 = mybir.dt.bfloat16
F8 = mybir.dt.float8e4
AF = mybir.ActivationFunctionType
OP = mybir.AluOpType
DR = mybir.MatmulPerfMode.DoubleRow

N_CORES = 8
B, T, D, DH, V = 2, 2048, 1024, 4096, 32000
TB = T // 4            # 512 tokens per core
VS = V // N_CORES      # 4000 vocab cols per core
VT = VS // 8           # 500 per n-tile
KC = D // 128          # 8 contraction chunks of d_model
HC = DH // 128         # 32 chunks of d_hidden
NTK = T // 128         # 16 key chunks (whole batch-sequence)
SCALE = 1.0 / 32.0     # 1/sqrt(D)

SX = 256.0             # fp8 scale for x
SW = 512.0             # fp8 scale for Wq/Wk/Wv
SQ = 64.0              # fp8 scale for q/k/v
SP = 16.0              # fp8 scale for exp(probs)
S_ACT = SCALE / (SQ * SQ)          # exp activation input scale (2^-17)
PROJ_IMM = SQ / (SX * SW)          # PSUM -> fp8 rescale for q/k/v (2^-11)
RS_IMM = 0.125 / SQ                # residual scale / v-scale (2^-9)
MSK_VIS = float(np.log(SP))        # post-scale bias for visible keys
MSK_NEG = -400.0                   # exp(-400) == 0: masked keys
XH = (KC // 2) * 128 * TB          # bf16 elems in one AllGather half

_STATE = {}
_NO_COLL = False   # timing/sim variant: skip collectives


def _trunk(nc, tc, io, dp, bounce1, bounce2, ag1, ag2, lg):
    """Token-parallel trunk; fills lg['xf'] via the split AllGather."""
    with tc.tile_pool(name="x1p", bufs=1) as x1p:
        x1T = x1p.tile([128, KC, TB], F32)
        x1b = x1p.tile([128, KC, TB], BF16)
        _trunk_attn(nc, tc, io, dp, x1T, x1b)
        _trunk_mlp(nc, tc, io, x1T, x1b, bounce1, bounce2, ag1, ag2, lg)


def _trunk_attn(nc, tc, io, dp, x1T, x1b):
    with tc.tile_pool(name="pA", bufs=1) as pA:
        kS = pA.tile([128, KC, T], F8)
        vS = pA.tile([128, NTK, D], F8)
        qS = pA.tile([128, KC, TB], F8)
        attnT = pA.tile([128, NTK, TB], F8)
        mskb = pA.tile([128, NTK], F32)
        ones_f = pA.tile([128, 2, 16], F32)
        ones8 = pA.tile([128, 2, 16], F8)
        nc.sync.dma_start(mskb[:], io["mskb"].ap())
        nc.vector.memset(ones_f[:], 1.0)
        nc.vector.tensor_copy(ones8[:], ones_f[:])

        # ---------- Q/K/V projections (fp8 DoubleRow) ----------
        with tc.tile_pool(name="proj", bufs=1) as pp, \
             tc.tile_pool(name="ps_pj", bufs=6, space="PSUM") as pspj:
            w8s = pp.tile([128, 3, KC, D], F8)
            x8s = pp.tile([128, 4, KC, TB], F8)
            # load order: own tokens + Wq first so Q-proj starts ASAP
            nc.sync.dma_start(x8s[:, 0], io["x8"].ap()[0])
            nc.sync.dma_start(w8s[:, 0], io["w8"].ap()[:, 0])
            nc.sync.dma_start(w8s[:, 2], io["w8"].ap()[:, 2])
            nc.sync.dma_start(w8s[:, 1], io["w8"].ap()[:, 1])
            for tb in range(1, 4):
                nc.sync.dma_start(x8s[:, tb], io["x8"].ap()[tb])
            # Q: own tokens only (rotated block 0)
            for m in range(KC):
                ps = pspj.tile([128, TB], F32, name="ps_pj")
                for kk in range(0, KC, 2):
                    nc.tensor.matmul(ps[:], w8s[:, 0, kk:kk + 2, bass.ts(m, 128)],
                                     x8s[:, 0, kk:kk + 2, :],
                                     start=(kk == 0), stop=(kk == KC - 2),
                                     perf_mode=DR)
                nc.vector.tensor_scalar_mul(qS[:, m, :], ps[:], PROJ_IMM)
            for tb in range(4):
                for i4 in range(4):
                    tc_ = 4 * tb + i4
                    for h in range(2):
                        ps = pspj.tile([128, 512], F32, name="ps_pj")
                        for kk in range(0, KC, 2):
                            nc.tensor.matmul(
                                ps[:], x8s[:, tb, kk:kk + 2, bass.ts(i4, 128)],
                                w8s[:, 2, kk:kk + 2, bass.ts(h, 512)],
                                start=(kk == 0), stop=(kk == KC - 2),
                                perf_mode=DR)
                        nc.vector.tensor_scalar_mul(
                            vS[:, tc_, bass.ts(h, 512)], ps[:], PROJ_IMM)
                for m in range(KC):
                    ps = pspj.tile([128, TB], F32, name="ps_pj")
                    for kk in range(0, KC, 2):
                        nc.tensor.matmul(
                            ps[:], w8s[:, 1, kk:kk + 2, bass.ts(m, 128)],
                            x8s[:, tb, kk:kk + 2, :],
                            start=(kk == 0), stop=(kk == KC - 2),
                            perf_mode=DR)
                    nc.scalar.activation(kS[:, m, bass.ts(tb, TB)], ps[:],
                                         AF.Copy, scale=PROJ_IMM)

        # ---------- attention (scores transposed: sT[tk, tq], fp8 DR) ------
        with tc.tile_pool(name="attn", bufs=1) as ap_, \
             tc.tile_pool(name="stp", bufs=3) as stp, \
             tc.tile_pool(name="ps_sc", bufs=4, space="PSUM") as ps_scp, \
             tc.tile_pool(name="ps_l", bufs=1, space="PSUM") as ps_lp, \
             tc.tile_pool(name="ps_av", bufs=2, space="PSUM") as ps_avp:
            x0own = ap_.tile([128, KC, TB], F32)
            mskd = ap_.tile([128, 4, TB], F32)
            nc.sync.dma_start(x0own[:],
                              io["x0own"].ap().rearrange("k p t -> p k t"))
            nc.sync.dma_start(mskd[:],
                              io["mskd"].ap().rearrange("c p t -> p c t"))
            ps_l = ps_lp.tile([1, TB], F32)
            for tkc in range(NTK):
                ps = ps_scp.tile([128, TB], F32, name="ps_sc")
                for kk in range(0, KC, 2):
                    nc.tensor.matmul(ps[:], kS[:, kk:kk + 2, bass.ts(tkc, 128)],
                                     qS[:, kk:kk + 2, :],
                                     start=(kk == 0), stop=(kk == KC - 2),
                                     perf_mode=DR)
                if tkc < 4:
                    # own-block diagonal: per-element mask (pre-scale units)
                    stmp = stp.tile([128, TB], F32, name="stmp")
                    nc.vector.tensor_tensor(out=stmp[:], in0=ps[:],
                                            in1=mskd[:, tkc, :], op=OP.add)
                    nc.scalar.activation(attnT[:, tkc, :], stmp[:], AF.Exp,
                                         scale=S_ACT)
                else:
                    # off-diagonal: whole chunk visible or masked (bias)
                    nc.scalar.activation(attnT[:, tkc, :], ps[:], AF.Exp,
                                         bias=mskb[:, tkc:tkc + 1],
                                         scale=S_ACT)
                if tkc % 2 == 1:
                    nc.tensor.matmul(ps_l[:], ones8[:, :, :1],
                                     attnT[:, tkc - 1:tkc + 1, :],
                                     start=(tkc == 1), stop=(tkc == NTK - 1),
                                     perf_mode=DR)

            # rs = 0.125 / (l * SQ), broadcast to all partitions via DRAM
            rs_row = ap_.tile([1, TB], F32)
            nc.vector.reciprocal(rs_row[:], ps_l[:])
            rs_row2 = ap_.tile([1, TB], F32)
            nc.vector.tensor_scalar_mul(rs_row2[:], rs_row[:], RS_IMM)
            rs_dram = dp.tile([1, TB], F32, name="rs_dram")
            nc.sync.dma_start(rs_dram[:], rs_row2[:])
            rs_b = ap_.tile([128, TB], F32)
            nc.sync.dma_start(rs_b[:], rs_dram[:].partition_broadcast(128))

            # oT[dv, tq] = V.T @ attnT ; x1 = x0 + rs * oT
            for m in range(KC):
                ps = ps_avp.tile([128, TB], F32, name="ps_av")
                for cc in range(0, NTK, 2):
                    nc.tensor.matmul(ps[:], vS[:, cc:cc + 2, bass.ts(m, 128)],
                                     attnT[:, cc:cc + 2, :],
                                     start=(cc == 0), stop=(cc == NTK - 2),
                                     perf_mode=DR)
                ot = stp.tile([128, TB], F32, name="stmp")
                nc.vector.tensor_tensor(out=ot[:], in0=ps[:], in1=rs_b[:],
                                        op=OP.mult)
                # f32 and bf16 residual adds run on separate engines
                nc.vector.tensor_tensor(out=x1T[:, m, :], in0=ot[:],
                                        in1=x0own[:, m, :], op=OP.add)
                nc.gpsimd.tensor_tensor(out=x1b[:, m, :], in0=ot[:],
                                        in1=x0own[:, m, :], op=OP.add)


def _trunk_mlp(nc, tc, io, x1T, x1b, bounce1, bounce2, ag1, ag2, lg):
    # ---------- MLP (bf16) + logits-weight prefetch + AllGather --------
    if True:
        with tc.tile_pool(name="mlp", bufs=1) as mp, \
             tc.tile_pool(name="w1p", bufs=6) as w1p, \
             tc.tile_pool(name="w2p", bufs=2) as w2p, \
             tc.tile_pool(name="ps_h", bufs=6, space="PSUM") as ps_hp:
            b1_s = mp.tile([128, HC], F32)
            b2_s = mp.tile([128, KC], F32)
            nc.sync.dma_start(b1_s[:], io["b1t"].ap())
            nc.sync.dma_start(b2_s[:], io["b2t"].ap())
            for n in range(8):
                nc.scalar.dma_start(
                    lg["bout"][:, n, :],
                    io["boutb"].ap()[n:n + 1, :].partition_broadcast(128))
            hT = mp.tile([128, HC, TB], BF16)
            for m in range(HC):
                w1t = w1p.tile([128, KC, 128], BF16, name="w1t")
                nc.sync.dma_start(
                    w1t[:],
                    io["w1b"].ap()[m].rearrange("p (k q) -> p k q", k=KC))
                ps = ps_hp.tile([128, TB], F32, name="ps_mlp")
                for k in range(KC):
                    nc.tensor.matmul(ps[:], w1t[:, k, :], x1b[:, k, :],
                                     start=(k == 0), stop=(k == KC - 1))
                nc.scalar.activation(hT[:, m, :], ps[:], AF.Gelu,
                                     bias=b1_s[:, m:m + 1], scale=1.0)
            lg["load_wot"](0)   # prefetch first logits weight tile
            x2T = mp.tile([128, KC, TB], BF16)
            TH = TB // 2
            # W2 split over token halves: the first AllGather (tokens 0:256,
            # ALL d-chunks) fires halfway through W2, letting the logits
            # phase start on token-half 0 while half 1 is still computing.
            for th in range(2):
                ts_ = slice(th * TH, (th + 1) * TH)
                for m in range(KC):
                    w2t = w2p.tile([128, HC, 128], BF16, name="w2t")
                    nc.scalar.dma_start(
                        w2t[:],
                        io["w2b"].ap()[m].rearrange("p (k q) -> p k q", k=HC))
                    ps = ps_hp.tile([128, TH], F32, name="ps_mlp")
                    for k in range(HC):
                        nc.tensor.matmul(ps[:], w2t[:, k, :], hT[:, k, ts_],
                                         start=(k == 0), stop=(k == HC - 1))
                    nc.vector.scalar_tensor_tensor(
                        out=x2T[:, m, ts_], in0=ps[:],
                        scalar=b2_s[:, m:m + 1],
                        in1=x1T[:, m, ts_], op0=OP.add, op1=OP.add)
                bnc, ag = (bounce1, ag1) if th == 0 else (bounce2, ag2)
                nc.scalar.dma_start(
                    bnc[:].rearrange("(k p t) -> p k t", k=KC, p=128),
                    x2T[:, :, ts_])
                if not _NO_COLL:
                    nc.gpsimd.collective_compute(
                        "AllGather", OP.bypass,
                        replica_groups=[list(range(N_CORES))],
                        ins=[bnc.opt()], outs=[ag.opt()])
                for r in range(N_CORES):
                    nc.gpsimd.dma_start(
                        lg["xf"][:, KC * r:KC * (r + 1), ts_],
                        ag[r].rearrange("(k p t) -> p k t", k=KC, p=128))


def _logits(nc, tc, io, lg, bare=False, nodma=False, n_count=8):
    """Vocab-parallel bf16 logits over the AllGathered hidden states."""
    out_d = io["logits"]
    xf, bout_s, wots, load_wot = lg["xf"], lg["bout"], lg["wots"], lg["load_wot"]
    with tc.tile_pool(name="outp", bufs=3) as outp, \
         tc.tile_pool(name="ps_lg", bufs=8, space="PSUM") as ps_lg:
        for n in range(n_count):
            if wots[n] is None:
                load_wot(n)
            if n < n_count - 1:
                load_wot(n + 1)
            wot = wots[n]
            for half in range(2):
                for r in range(N_CORES):
                    ot = outp.tile([128, 2, VT], BF16, name="og")
                    for i, t4 in enumerate((2 * half, 2 * half + 1)):
                        ps = ps_lg.tile([128, VT], F32, name="ps_g")
                        for k in range(KC):
                            nc.tensor.matmul(
                                ps[:], xf[:, KC * r + k, bass.ts(t4, 128)],
                                wot[:, k, :],
                                start=(k == 0), stop=(k == KC - 1))
                        if not bare:
                            nc.vector.tensor_tensor(
                                out=ot[:, i, :], in0=ps[:],
                                in1=bout_s[:, n, :], op=OP.add)
                    if bare:
                        nc.vector.tensor_tensor(out=ot[:, 0, :], in0=ps[:],
                                                in1=bout_s[:, n, :],
                                                op=OP.add)
                    if not bare and not nodma:
                        nc.scalar.dma_start(out_d.ap()[r, n, half], ot[:])
        if bare or nodma:
            nc.scalar.dma_start(out_d.ap()[0, 0, 0], ot[:])


def _build(repeat=1, phases="full"):
    nc = bacc.Bacc("TRN2", target_bir_lowering=False, debug=False,
                   num_devices=N_CORES)

    # ---- kernel I/O (per-core shards prepared on host) ----
    io = {}
    def inp(name, shape, dt=F32):
        io[name] = nc.dram_tensor(name, shape, dt, kind="ExternalInput")
    inp("x8", [4, 128, KC, TB], F8)
    inp("x0own", [KC, 128, TB])
    inp("w8", [128, 3, KC, D], F8)
    inp("mskd", [4, 128, TB])
    inp("mskb", [128, NTK])
    inp("w1b", [HC, 128, KC * 128], BF16)
    inp("b1t", [128, HC])
    inp("w2b", [KC, 128, HC * 128], BF16)
    inp("b2t", [128, KC])
    inp("woutb", [8, 128, KC * VT], BF16)
    inp("boutb", [8, VT], BF16)
    io["logits"] = nc.dram_tensor("logits", [N_CORES, 8, 2, 128, 2, VT],
                                  BF16, kind="ExternalOutput")

    with tile.TileContext(nc) as tc:
        with tc.tile_pool(name="dram", bufs=1, space="DRAM") as dp:
            for _ in range(repeat):  # repeat>1 is a timing-only variant
                bounce1 = dp.tile([XH], BF16, name="bounce1")
                bounce2 = dp.tile([XH], BF16, name="bounce2")
                ag1 = dp.tile([N_CORES, XH], BF16, name="ag1",
                              addr_space="Shared")
                ag2 = dp.tile([N_CORES, XH], BF16, name="ag2",
                              addr_space="Shared")
                with tc.tile_pool(name="lgP", bufs=1) as lgP, \
                     tc.tile_pool(name="wop", bufs=2) as wop:
                    wots = [None] * 8

                    def load_wot(n, _w=wop, _io=io, _ws=wots):
                        t = _w.tile([128, KC, VT], BF16, name="wot")
                        nc.scalar.dma_start(
                            t[:],
                            _io["woutb"].ap()[n].rearrange(
                                "p (k v) -> p k v", k=KC))
                        _ws[n] = t

                    xf = lgP.tile([128, N_CORES * KC, TB], BF16, name="xf")
                    bout_s = lgP.tile([128, 8, VT], BF16, name="bout_s")
                    lg = {
                        "xf": xf,
                        "bout": bout_s,
                        "wots": wots,
                        "load_wot": load_wot,
                    }
                    if phases in ("full", "trunk"):
                        _trunk(nc, tc, io, dp, bounce1, bounce2, ag1, ag2, lg)
                    if phases == "logits":
                        # standalone logits timing: feed xf from (garbage) ag
                        for n in range(8):
                            nc.scalar.dma_start(
                                bout_s[:, n, :],
                                io["boutb"].ap()[n:n + 1, :]
                                .partition_broadcast(128))
                        TH = TB // 2
                        for r in range(N_CORES):
                            nc.sync.dma_start(
                                xf[:, KC * r:KC * (r + 1), :TH],
                                ag1[r].rearrange("(k p t) -> p k t",
                                                 k=KC, p=128))
                            nc.sync.dma_start(
                                xf[:, KC * r:KC * (r + 1), TH:],
                                ag2[r].rearrange("(k p t) -> p k t",
                                                 k=KC, p=128))
                    if phases in ("full", "logits"):
                        _logits(nc, tc, io, lg)
                    if phases in ("full-nodma", "full-bare",
                                  "full-bare-halfn"):
                        _trunk(nc, tc, io, dp, bounce1, bounce2, ag1, ag2, lg)
                        _logits(nc, tc, io, lg,
                                bare=phases.startswith("full-bare"),
                                nodma=(phases == "full-nodma"),
                                n_count=(4 if phases.endswith("halfn") else 8))
                    if phases == "logits-bare":
                        for n in range(8):
                            nc.scalar.dma_start(
                                bout_s[:, n, :],
                                io["boutb"].ap()[n:n + 1, :]
                                .partition_broadcast(128))
                        TH = TB // 2
                        for r in range(N_CORES):
                            nc.sync.dma_start(
                                xf[:, KC * r:KC * (r + 1), :TH],
                                ag1[r].rearrange("(k p t) -> p k t",
                                                 k=KC, p=128))
                            nc.sync.dma_start(
                                xf[:, KC * r:KC * (r + 1), TH:],
                                ag2[r].rearrange("(k p t) -> p k t",
                                                 k=KC, p=128))
                        _logits(nc, tc, io, lg, bare=True)
                    if phases == "logits-nodma":
                        for n in range(8):
                            nc.scalar.dma_start(
                                bout_s[:, n, :],
                                io["boutb"].ap()[n:n + 1, :]
                                .partition_broadcast(128))
                        TH = TB // 2
                        for r in range(N_CORES):
                            nc.sync.dma_start(
                                xf[:, KC * r:KC * (r + 1), :TH],
                                ag1[r].rearrange("(k p t) -> p k t",
                                                 k=KC, p=128))
                            nc.sync.dma_start(
                                xf[:, KC * r:KC * (r + 1), TH:],
                                ag2[r].rearrange("(k p t) -> p k t",
                                                 k=KC, p=128))
                        _logits(nc, tc, io, lg, nodma=True)

    nc.compile()
    return nc


def _prep_shared(Wq, Wk, Wv, W1, b1, W2, b2, pos_emb):
    f = np.float32
    bf = ml_dtypes.bfloat16
    f8 = ml_dtypes.float8_e4m3
    sh = {}
    w3 = np.stack([Wq, Wk, Wv]).astype(f)                     # [3, D, D]
    sh["w8"] = np.ascontiguousarray(
        (w3.reshape(3, KC, 128, D).transpose(2, 0, 1, 3) * SW)).astype(f8)
    sh["w1b"] = np.ascontiguousarray(
        W1.reshape(KC, 128, HC, 128).transpose(2, 1, 0, 3).reshape(
            HC, 128, KC * 128)).astype(bf)
    sh["b1t"] = np.ascontiguousarray(b1.reshape(HC, 128).T, dtype=f)
    sh["w2b"] = np.ascontiguousarray(
        W2.reshape(HC, 128, KC, 128).transpose(2, 1, 0, 3).reshape(
            KC, 128, HC * 128)).astype(bf)
    sh["b2t"] = np.ascontiguousarray(b2.reshape(KC, 128).T, dtype=f)

    # own-block diagonal masks (core-independent after rotation):
    # key (tkc*128+rr) visible to query cc iff tkc*128+rr <= cc.
    rr = np.arange(128)[:, None]
    cc = np.arange(TB)[None, :]
    mskd = np.empty((4, 128, TB), dtype=f)
    for tkc in range(4):
        mskd[tkc] = np.where(tkc * 128 + rr <= cc,
                             MSK_VIS / S_ACT, MSK_NEG / S_ACT)
    sh["mskd"] = mskd

    # per-j off-diagonal chunk biases (post-scale units) and block orders
    orders = [[(j + i) % 4 for i in range(4)] for j in range(4)]
    mskbs = []
    for j in range(4):
        v = np.empty(NTK, dtype=f)
        v[:4] = 0.0   # unused (diagonal chunks use mskd)
        for tkc in range(4, NTK):
            br = orders[j][tkc // 4]
            v[tkc] = MSK_VIS if br < j else MSK_NEG
        mskbs.append(np.ascontiguousarray(np.tile(v[None, :], (128, 1))))
    pos = np.asarray(pos_emb[:T], dtype=f)
    return sh, orders, mskbs, pos


def make_in_maps(idx, tok_emb, pos_emb, Wq, Wk, Wv, W1, b1, W2, b2,
                 Wout, bout):
    f = np.float32
    bf = ml_dtypes.bfloat16
    f8 = ml_dtypes.float8_e4m3
    tok_emb = np.asarray(tok_emb, dtype=f)
    idx = np.asarray(idx)
    sh, orders, mskbs, pos = _prep_shared(
        np.asarray(Wq, f), np.asarray(Wk, f), np.asarray(Wv, f),
        np.asarray(W1, f), np.asarray(b1, f), np.asarray(W2, f),
        np.asarray(b2, f), np.asarray(pos_emb, f))
    Wout = np.asarray(Wout, f)
    bout = np.asarray(bout, f)

    in_maps = []
    for c in range(N_CORES):
        b, j = c // 4, c % 4
        x_full = tok_emb[np.asarray(idx[b], dtype=np.int64)] + pos  # [T, D]
        xr = np.concatenate([x_full[TB * br:TB * (br + 1)]
                             for br in orders[j]])
        m = dict(sh)
        m["x8"] = np.ascontiguousarray(
            (xr.reshape(4, TB, KC, 128).transpose(0, 3, 2, 1) * SX)).astype(f8)
        m["x0own"] = np.ascontiguousarray(
            xr[:TB].T.reshape(KC, 128, TB), dtype=f)
        m["mskb"] = mskbs[j]
        ws = Wout[:, VS * c:VS * (c + 1)]
        m["woutb"] = np.ascontiguousarray(
            ws.reshape(KC, 128, 8, VT).transpose(2, 1, 0, 3).reshape(
                8, 128, KC * VT)).astype(bf)
        m["boutb"] = np.ascontiguousarray(
            bout[VS * c:VS * (c + 1)].reshape(8, VT)).astype(bf)
        in_maps.append(m)
    return in_maps


def kernel(idx, tok_emb, pos_emb, Wq, Wk, Wv, W1, b1, W2, b2, Wout, bout):
    if "nc" not in _STATE:
        _STATE["nc"] = _build()
    nc = _STATE["nc"]

    in_maps = make_in_maps(idx, tok_emb, pos_emb, Wq, Wk, Wv, W1, b1, W2,
                           b2, Wout, bout)
    res = bass_utils.run_bass_kernel_spmd(nc, in_maps,
                                          core_ids=list(range(N_CORES)))
    _STATE["last_results"] = res

    out = np.empty((B * T, V), dtype=np.float32)
    for c in range(N_CORES):
        lg = np.asarray(res.results[c]["logits"], dtype=np.float32)
        # [r, n, half, p, i, v]: row r*512 + half*256 + i*128 + p
        out[:, VS * c:VS * (c + 1)] = (
            lg.transpose(0, 2, 4, 3, 1, 5).reshape(B * T, VS))
    return out.reshape(B, T, V)
